# revision 12
# baseline (speedup 1.0000x reference)
"""Trainium2 Bass kernel for CorrespondenceGenerationArch (patch cross-correlation + argmax).

Math: channel-normalize both (256,72,72) feature maps, extract 3x3 patches
(4900 x 2304 each), corr = pin @ pref.T (4900x4900, K=2304), per-row argmax
(first occurrence), then index -> flow arithmetic to a (1,9,288,288,2) output.
Only the argmax feeds the output; the pref row-normalization is a uniform
scale (patch row norms are exactly 3 after channel norm) so it cannot change
the argmax.

Distribution: input-patch rows sharded across 8 cores (9 of 70 y-rows each,
no collectives). Each core computes its 630x4900 slab of the correlation as
5 M-tiles x 10 N-chunks x 9 K-step accumulated fp8 DoubleRow matmuls (K=256
per step, fp32 PSUM accumulation); the moving operand is a shifted view of
the ref image resident in SBUF (ref patches are never materialized).
Weights are kept stationary across groups of 4 N-chunks (kpair-outer order)
so weight loads amortize over 4x the streaming. The scalar engine copies
each finished PSUM chunk to a contiguous fp16 row buffer in SBUF; one DVE
max8/max_index pair per M-tile then extracts the top-8 values + global
indices over the whole 4900-wide row. The host exact-rescores the few
candidates within the fp8 error envelope of each row's best and resolves
coverage-flagged rows with an exact f64 recheck, then assembles the output.
"""

import os
import numpy as np
import ml_dtypes

C = 256
H = W = 72
HO = WO = 70
NPATCH = HO * WO            # 4900
NCORES = 8
YROWS = 9                   # y-rows of patches per core (8*9 = 72 >= 70)
MROWS = YROWS * WO          # 630 valid patch rows per core
MT = 128                    # M-tile (full PE width)
NMT = 5
MPAD = NMT * MT             # 640 rows incl. 10 zero-pad rows
NCH = 490                   # N-chunk (10 * 490 = 4900), 7 v-rows of 70
NNCH = 10
KSTEPS = 18                 # (dy,dx) x channel-half
KPAIRS = 9                  # fp8 DoubleRow: one step per (dy,dx), K=256
GROUPS = ((0, 4), (4, 8), (8, 10))  # N-chunk groups sharing stationary weights
SCALE = np.float32(64.0)    # fp8 input scale (|x|<=1 -> <=64, e4m3 max 448)
SCALE2 = SCALE * SCALE
ERRB = np.float32(0.06)     # fp8 corr |err| bound (measured max 0.038)

_PROGRAM_CACHE = {}
LAST_RESULTS = None


def _build_program(loop_r=1, psum_bufs=4, out_bufs=3):
    import concourse.tile as tile
    from concourse import bacc, mybir

    f32 = mybir.dt.float32
    f16 = mybir.dt.float16
    f8 = mybir.dt.float8e4
    u16 = mybir.dt.uint16
    DR = mybir.MatmulPerfMode.DoubleRow

    nc = bacc.Bacc(
        "TRN2", target_bir_lowering=False, debug=False, num_devices=NCORES
    )
    finp_d = nc.dram_tensor(
        "finp", (NMT, 128, KSTEPS, MT), f8, kind="ExternalInput"
    ).ap()
    fref_d = nc.dram_tensor("fref", (128, 2, H, W), f8, kind="ExternalInput").ap()
    vals_d = nc.dram_tensor(
        "vals8", (MPAD, NNCH // 2, 8), f16, kind="ExternalOutput"
    ).ap()
    idx_d = nc.dram_tensor(
        "idx8", (MPAD, NNCH // 2, 8), u16, kind="ExternalOutput"
    ).ap()

    with tile.TileContext(nc) as tc:
        with (
            tc.tile_pool(name="const", bufs=1) as cpool,
            tc.tile_pool(name="row", bufs=2) as rpool,
            tc.tile_pool(name="outs", bufs=out_bufs) as opool,
            tc.tile_pool(name="psum", bufs=psum_bufs, space="PSUM") as ppool,
        ):
            # Input DMAs, finest-consumer-first so the PE can start early:
            # one finp slab per M-tile plus the whole ref image in one tile
            # (every shifted moving-operand view slices it in place).
            NPAIR = NNCH // 2           # 5 pair-chunks of 980 ref patches
            PGROUPS = ((0, 2), (2, 4), (4, 5))
            def body(_i=None):
                finp_sb = []
                for m in range(NMT):
                    t = cpool.tile([128, KSTEPS, MT], f8, tag=f"finp{m}",
                                   name=f"finp_{m}")
                    finp_sb.append(t)
                band = cpool.tile([128, 2, H, W], f8, tag="band", name="band")
                nc.sync.dma_start(finp_sb[0][:], finp_d[0])
                nc.sync.dma_start(band[:], fref_d[:])
                for m in range(1, NMT):
                    nc.sync.dma_start(finp_sb[m][:], finp_d[m])

                for m in range(NMT):
                    ct = rpool.tile([MT, NNCH * NCH], f16, tag="ct", name=f"ct_{m}")
                    vb = opool.tile([MT, NPAIR, 8], f16, tag="vb",
                                    name=f"vb_{m}")
                    ib = opool.tile([MT, NPAIR, 8], u16, tag="ib",
                                    name=f"ib_{m}")
                    for p0, p1 in PGROUPS:
                        pts = [
                            ppool.tile([MT, 2, 512], f32, tag="pt",
                                       name=f"pt_{m}_{p}")
                            for p in range(p0, p1)
                        ]
                        # kpair-outer: each stationary weight streams all
                        # chunks of the group before the next weight load.
                        for t in range(KPAIRS):
                            dy, dx = divmod(t, 3)
                            for j, p in enumerate(range(p0, p1)):
                                for s in range(2):
                                    n = 2 * p + s
                                    nc.tensor.matmul(
                                        pts[j][:, s, :NCH],
                                        finp_sb[m][:, 2 * t : 2 * t + 2, :],
                                        band[:, :, 7 * n + dy : 7 * n + dy + 7,
                                             dx : dx + WO],
                                        start=(t == 0),
                                        stop=(t == KPAIRS - 1),
                                        perf_mode=DR,
                                    )
                        for j, p in enumerate(range(p0, p1)):
                            for s in range(2):
                                n = 2 * p + s
                                nc.scalar.copy(
                                    ct[:, n * NCH : (n + 1) * NCH],
                                    pts[j][:, s, :NCH],
                                )
                    for p in range(NPAIR):
                        sl = ct[:, 2 * p * NCH : (2 * p + 2) * NCH]
                        nc.vector.max(vb[:, p, :], sl)
                        nc.vector.max_index(ib[:, p, :], vb[:, p, :], sl)
                    nc.sync.dma_start(vals_d[MT * m : MT * (m + 1)], vb[:])
                    nc.sync.dma_start(idx_d[MT * m : MT * (m + 1)], ib[:])

            if loop_r == 1:
                body()
            else:
                with tc.For_i(0, loop_r, 1):
                    body()

    _dedupe_ldweights(nc)
    nc.compile()
    return nc


def _dedupe_ldweights(nc):
    """Drop InstLdweights that reload the PE's already-loaded weights.

    The kpair-outer loop issues runs of up to 4 matmuls sharing one
    stationary tile slice, but each InstMatmult is emitted with its own
    self-load. Within a block, a load whose (weights AP, perf_mode) equals
    the previous load's is redundant: weights persist in the PE array across
    matmuls, nothing references the duplicate by name, and its sync deps are
    identical to the surviving load's (same producer DMA). Only provably
    identical duplicates are removed.
    """
    for f in nc.m.functions:
        for blk in f.blocks:
            insts = list(blk.instructions)
            used = set()
            for x in insts:
                used.update(x.sync_dependency_names())
                used.update(x.nosync_dependency_names())
            kept = []
            cur = None  # (key, deps) of the load currently in the PE array
            removed = 0
            for x in insts:
                if type(x).__name__ == "InstLdweights":
                    key = (str(x.ins[0]), str(x.perf_mode), str(x.is_transpose))
                    deps = (
                        tuple(x.sync_dependency_names()),
                        tuple(x.nosync_dependency_names()),
                    )
                    if (
                        cur is not None
                        and cur == (key, deps)
                        and x.name not in used
                    ):
                        removed += 1
                        continue
                    cur = (key, deps)
                kept.append(x)
            if removed:
                blk.instructions = kept


def _get_program():
    if "nc" not in _PROGRAM_CACHE:
        _PROGRAM_CACHE["nc"] = _build_program()
    return _PROGRAM_CACHE["nc"]


def _chan_norm(f):
    n = np.sqrt(np.sum(f * f, axis=0, keepdims=True, dtype=np.float32),
                dtype=np.float32)
    return (f / np.maximum(n, np.float32(1e-12))).astype(np.float32)


def _host_inputs(fin_n, fref_n):
    """Per-core finp patch slabs (NMT,128,KSTEPS,MT) + shared fref pack, fp8."""
    fref_pack = np.ascontiguousarray(
        (fref_n * SCALE).reshape(2, 128, H, W).transpose(1, 0, 2, 3)
    ).astype(ml_dtypes.float8_e4m3)
    fin_pad = np.zeros((C, H + 2, W), np.float32)
    fin_pad[:, :H, :] = fin_n * SCALE
    in_maps = []
    for c in range(NCORES):
        y0 = YROWS * c
        ks = []
        for dy in range(3):
            for dx in range(3):
                for ch in range(2):
                    a = fin_pad[
                        ch * 128 : (ch + 1) * 128,
                        y0 + dy : y0 + dy + YROWS,
                        dx : dx + WO,
                    ].reshape(128, MROWS)
                    ks.append(a)
        finp = np.zeros((128, KSTEPS, MPAD), np.float32)
        finp[:, :, :MROWS] = np.stack(ks, axis=1)  # (128, 18, 630)
        finp = np.ascontiguousarray(
            finp.reshape(128, KSTEPS, NMT, MT).transpose(2, 0, 1, 3)
        ).astype(ml_dtypes.float8_e4m3)
        in_maps.append({"finp": finp, "fref": fref_pack})
    return in_maps


def _patches(f):
    """(4900, 2304) patch matrix; k order (dy,dx,c) - irrelevant for dots."""
    cols = [
        f[:, dy : dy + HO, dx : dx + WO].reshape(C, -1)
        for dy in range(3)
        for dx in range(3)
    ]
    return np.concatenate(cols, axis=0).T


def _assemble(max_idx):
    max_idx = max_idx.reshape(HO, WO)
    flow_w = (max_idx % WO).astype(np.float32)
    flow_h = (max_idx // WO).astype(np.float32)
    gx = np.arange(WO, dtype=np.float32)[None, :]
    gy = np.arange(HO, dtype=np.float32)[:, None]
    flow = np.stack((flow_w - gx, flow_h - gy), axis=2)[None]  # (1,70,70,2)
    flow = np.pad(flow, ((0, 0), (0, 2), (0, 2), (0, 0)))
    off = np.repeat(np.repeat(flow, 4, axis=1), 4, axis=2) * np.float32(4.0)
    outs = []
    for i in range(3):
        for j in range(3):
            sh, sw = i * 4, j * 4
            outs.append(
                np.pad(
                    off[:, : 4 * H - sh, : 4 * W - sw, :],
                    ((0, 0), (sh, 0), (sw, 0), (0, 0)),
                )
            )
    return np.concatenate(outs, axis=0)[None]  # (1,9,288,288,2)


def kernel(dense_features1, dense_features2, img_ref_hr):
    global LAST_RESULTS
    # No NTFF profile hook is available under this axon client; a set
    # BASS_TRACE would send run_bass_kernel_spmd down an import that fails.
    os.environ["BASS_NEVER_TRACE"] = "1"
    from concourse.bass_utils import run_bass_kernel_spmd

    assert dense_features1.shape == (1, C, H, W), dense_features1.shape
    f1 = np.asarray(dense_features1, np.float32)[0]
    f2 = np.asarray(dense_features2, np.float32)[0]
    fin_n = _chan_norm(f1)
    fref_n = _chan_norm(f2)

    nc = _get_program()
    in_maps = _host_inputs(fin_n, fref_n)
    res = run_bass_kernel_spmd(nc, in_maps, list(range(NCORES)))
    LAST_RESULTS = res

    NP2 = NNCH // 2
    vals = np.empty((NPATCH, NP2, 8), np.float32)
    lidx = np.empty((NPATCH, NP2, 8), np.int64)
    for c in range(NCORES):
        y0 = YROWS * c
        nvalid = max(0, min(YROWS, HO - y0)) * WO
        if nvalid == 0:
            continue
        vals[y0 * WO : y0 * WO + nvalid] = (
            res.results[c]["vals8"][:nvalid].astype(np.float32)
        )
        lidx[y0 * WO : y0 * WO + nvalid] = res.results[c]["idx8"][:nvalid].astype(
            np.int64
        )

    gidx = lidx + (np.arange(NP2, dtype=np.int64) * (2 * NCH))[None, :, None]
    flatv = vals.reshape(NPATCH, NP2 * 8) / SCALE2  # unscaled fp8 corr
    flatg = gidx.reshape(NPATCH, NP2 * 8)
    pin = _patches(fin_n)
    pref = _patches(fref_n)

    # Exact rescore of every candidate that could win given the fp8 error
    # envelope (typically a handful per row); f64 accumulation.
    best8 = flatv.max(axis=1)
    rows, slots = np.nonzero(flatv >= best8[:, None] - 2.0 * ERRB)
    cols = flatg[rows, slots]
    E = np.einsum("ij,ij->i", pin[rows], pref[cols], dtype=np.float64)
    best_exact = np.full(NPATCH, -np.inf)
    np.maximum.at(best_exact, rows, E)
    winner_mask = E >= best_exact[rows]  # exact ties -> keep all, then min idx
    max_idx = np.full(NPATCH, np.int64(NPATCH), np.int64)
    np.minimum.at(max_idx, rows[winner_mask], cols[winner_mask])

    # Coverage: a non-candidate in pair-chunk p is bounded by that chunk's
    # 8th fp8 value + ERRB; recheck rows where that bound beats our best.
    v8max = (vals[:, :, 7].max(axis=1)) / SCALE2
    amb = best_exact < (v8max + ERRB)
    if amb.any():
        pnorm = np.sqrt(np.sum(pref * pref, axis=1, keepdims=True, dtype=np.float32),
                        dtype=np.float32)
        prefn = pref / (pnorm + np.float32(1e-5))
        sub = pin[amb].astype(np.float64) @ prefn.T.astype(np.float64)
        max_idx[amb] = np.argmax(sub, axis=1)

    return _assemble(max_idx).astype(np.float32)
